# revision 20
# baseline (speedup 1.0000x reference)
"""MoE kernel for 8-core TRN2 (Bass/Tile), expert-parallel with sparse
token dispatch.

Per core e (of 8):
  - Routed expert e computed SPARSELY: on-device fp32 top-2 routing
    builds a compact token list (capacity C=1600, 200 per 512-token
    chunk), tokens are gathered by indirect DMA, PE-transposed, and run
    through the expert FFN in float32r; compact outputs ye + token
    indices are returned and the host scatters them back.
  - Shared expert is tensor-parallel: core e owns columns/rows
    [e*352:(e+1)*352] of Ws_* and computes its dense partial y.
  - Router must match the fp32 reference top-2 selection, so it runs as
    fp32 matmuls, packed 4-wide into PE column groups (M=8 each) and
    combined with a small fp32 matmul.

Host: out = sum_e y_e  +  scatter_add_e(ye_e at idx_e).
"""

import os
from contextlib import ExitStack

import numpy as np

import concourse.bass as bass
import concourse.mybir as mybir
import concourse.tile as tile
from concourse import bacc
from concourse.alu_op_type import AluOpType
from concourse.bass_utils import run_bass_kernel_spmd
from concourse.masks import make_identity

F32 = mybir.dt.float32
F32R = mybir.dt.float32r
U32 = mybir.dt.uint32
AF = mybir.ActivationFunctionType
AX = mybir.AxisListType

P = 128
E = 8
D = 2048
DE = 1408
DS = 2816
DSH = DS // E            # 352
B, S = 2, 2048
T = B * S                # 4096

KD = D // P              # 16
TCH = 512
NCH = T // TCH           # 8
MT = TCH // P            # 4
ND = D // 512            # 4
SH_MS = [P, P, DSH - 2 * P]
NME = DE // P            # 11

C8 = 184                 # per-chunk expert capacity
C = C8 * NCH             # 1600
QS = [C // 4] * 4        # 400 each (>=256 keeps f32r at full rate)

_CACHED = {}


def _build_program():
    nc = bacc.Bacc("TRN2", target_bir_lowering=False, debug=False, num_devices=E)

    x_d = nc.dram_tensor("x", [T + 1, D], F32R, kind="ExternalInput")   # row T = 0
    xT_d = nc.dram_tensor("xT", [D, T], F32, kind="ExternalInput")
    wg_d = nc.dram_tensor("wg", [D, DE], F32R, kind="ExternalInput")
    wu_d = nc.dram_tensor("wu", [D, DE], F32R, kind="ExternalInput")
    wd_d = nc.dram_tensor("wd", [DE, D], F32R, kind="ExternalInput")
    wsg_d = nc.dram_tensor("wsg", [D, DSH], F32R, kind="ExternalInput")
    wsu_d = nc.dram_tensor("wsu", [D, DSH], F32R, kind="ExternalInput")
    wsd_d = nc.dram_tensor("wsd", [DSH, D], F32R, kind="ExternalInput")
    wr_d = nc.dram_tensor("wr", [D, E], F32, kind="ExternalInput")
    esel_d = nc.dram_tensor("esel", [P, E], F32, kind="ExternalInput")
    ltri_d = nc.dram_tensor("ltri", [P, P], F32, kind="ExternalInput")  # L[q,p]=1 if q<=p
    m4_d = nc.dram_tensor("m4", [P, E], F32, kind="ExternalInput")      # col-group combine
    y_d = nc.dram_tensor("y", [T, D], F32, kind="ExternalOutput")
    ye_d = nc.dram_tensor("ye", [C, D], F32, kind="ExternalOutput")
    idx_d = nc.dram_tensor("idx", [1, C], U32, kind="ExternalOutput")

    xT_r = xT_d[:].rearrange("(k p) t -> p k t", p=P)
    wg_r = wg_d[:].rearrange("(k p) m -> p k m", p=P)
    wu_r = wu_d[:].rearrange("(k p) m -> p k m", p=P)
    wd_r = wd_d[:].rearrange("(k p) m -> p k m", p=P)

    with tile.TileContext(nc) as tc, ExitStack() as ctx:
        dram = ctx.enter_context(tc.tile_pool(name="dram", bufs=1, space="DRAM"))
        cc_buf = dram.tile([1, C], F32)
        xg_buf = dram.tile([C, D], F32R)

        const = ctx.enter_context(tc.tile_pool(name="const", bufs=1))
        identF = const.tile([P, P], F32)
        make_identity(nc, identF[:])
        identR = const.tile([P, P], F32R)
        nc.vector.tensor_copy(out=identR[:], in_=identF[:])
        esel_sb = const.tile([P, E], F32)
        nc.gpsimd.dma_start(out=esel_sb[:], in_=esel_d[:])
        ltri = const.tile([P, P], F32)
        nc.gpsimd.dma_start(out=ltri[:], in_=ltri_d[:])
        m4_sb = const.tile([P, E], F32)
        nc.gpsimd.dma_start(out=m4_sb[:], in_=m4_d[:])
        ones = const.tile([P, 1], F32)
        nc.vector.memset(ones[:], 1.0)
        wr_sb = []
        for k in range(KD):
            t = const.tile([P, E], F32, tag=f"wr{k}", name=f"wr{k}")
            nc.gpsimd.dma_start(out=t[:], in_=wr_d[k * P:(k + 1) * P, :])
            wr_sb.append(t)
        with tc.tile_pool(name="initp", bufs=1) as initp:
            initt = initp.tile([1, C], U32)
            nc.vector.memset(initt[:], T)
            nc.sync.dma_start(out=idx_d[:], in_=initt[:])
            initc = initp.tile([1, C], F32)
            nc.vector.memset(initc[:], 0.0)
            nc.sync.dma_start(out=cc_buf[:], in_=initc[:])
        tok_all = const.tile([P, T // P], U32)
        nc.gpsimd.iota(tok_all[:], pattern=[[P, T // P]], base=0, channel_multiplier=1)

        # ---------------- phase 1: routing + shared expert ----------------
        with ExitStack() as actx, nc.named_scope("phase1"):
            swp = actx.enter_context(tc.tile_pool(name="swp", bufs=1))
            wsg_sb = swp.tile([P, KD * DSH], F32R)
            wsg_v = wsg_sb[:].rearrange("p (k m) -> p k m", k=KD)
            nc.gpsimd.dma_start(out=wsg_v,
                                in_=wsg_d[:].rearrange("(k p) m -> p k m", p=P))
            wsu_sb = swp.tile([P, KD * DSH], F32R)
            wsu_v = wsu_sb[:].rearrange("p (k m) -> p k m", k=KD)
            nc.gpsimd.dma_start(out=wsu_v,
                                in_=wsu_d[:].rearrange("(k p) m -> p k m", p=P))
            wsd_sb = []
            for k3 in range(3):
                sz = SH_MS[k3]
                t = swp.tile([P, D], F32R, tag=f"wsd{k3}", name=f"wsd{k3}")
                nc.gpsimd.dma_start(out=t[:sz], in_=wsd_d[k3 * P:k3 * P + sz, :])
                wsd_sb.append(t)
            s4 = swp.tile([P, TCH], F32)
            nc.vector.memset(s4[:], 0.0)

            rps_p = actx.enter_context(tc.tile_pool(name="rps", bufs=1, space="PSUM"))
            rt_p = actx.enter_context(tc.tile_pool(name="rtp", bufs=2, space="PSUM"))
            pos_p = actx.enter_context(tc.tile_pool(name="posp", bufs=1, space="PSUM"))
            sp_p = actx.enter_context(tc.tile_pool(name="spp", bufs=2, space="PSUM"))
            yp_p = actx.enter_context(tc.tile_pool(name="ypp", bufs=2, space="PSUM"))
            xfp = actx.enter_context(tc.tile_pool(name="xfp", bufs=1))
            gpp = actx.enter_context(tc.tile_pool(name="gpp", bufs=2))
            # xg staging single-buffered (latency-tolerant)
            xrp = actx.enter_context(tc.tile_pool(name="xrp", bufs=2))
            rout = actx.enter_context(tc.tile_pool(name="rout", bufs=2))
            hsp = actx.enter_context(tc.tile_pool(name="hsp", bufs=2))
            ysp = actx.enter_context(tc.tile_pool(name="ysp", bufs=2))

            def emit_pos_and_scatter(pc, m_all, cv_all):
                """Positions + compact scatters for chunk pc (runs one chunk late
                so the PE-side ppre matmul never waits on the softmax chain)."""
                ppre = pos_p.tile([P, 2 * MT], F32, tag="ppre")
                nc.tensor.matmul(ppre[:, :MT], lhsT=ltri[:], rhs=m_all[:],
                                 start=True, stop=True)
                nc.tensor.matmul(ppre[:1, MT:], lhsT=ones[:], rhs=m_all[:],
                                 start=True, stop=True)
                pose = rout.tile([P, MT], F32, tag="pose")
                nc.vector.tensor_tensor(out=pose[:], in0=ppre[:, :MT], in1=m_all[:],
                                        op=AluOpType.subtract)
                cnt = rout.tile([1, MT], F32, tag="cnt")
                nc.vector.tensor_copy(out=cnt[:], in_=ppre[0:1, MT:])
                zero1 = rout.tile([1, MT], F32, tag="zero1")
                nc.vector.memset(zero1[:], 0.0)
                incl = rout.tile([1, MT], F32, tag="incl")
                nc.vector.tensor_tensor_scan(incl[:], cnt[:], zero1[:], 0.0,
                                             op0=AluOpType.add, op1=AluOpType.add)
                base = rout.tile([1, MT], F32, tag="base")
                nc.vector.tensor_sub(base[:], incl[:], cnt[:])
                base_b = rout.tile([P, MT], F32, tag="base_b")
                nc.gpsimd.partition_broadcast(base_b[:], base[:])
                nc.vector.tensor_add(pose[:], pose[:], base_b[:])
                pmask = rout.tile([P, MT], F32, tag="pmask")
                nc.vector.tensor_scalar(pmask[:], m_all[:], float(-C),
                                        float(C + pc * C8),
                                        op0=AluOpType.mult, op1=AluOpType.add)
                nc.vector.tensor_add(pmask[:], pmask[:], pose[:])
                posi = rout.tile([P, MT], U32, tag="posi")
                nc.vector.tensor_copy(out=posi[:], in_=pmask[:])
                for j in range(MT):
                    nc.gpsimd.indirect_dma_start(
                        out=idx_d[0, :, None],
                        out_offset=bass.IndirectOffsetOnAxis(ap=posi[:, j:j + 1],
                                                             axis=0),
                        in_=tok_all[:, pc * MT + j:pc * MT + j + 1], in_offset=None,
                        bounds_check=C - 1, oob_is_err=False)
                    nc.gpsimd.indirect_dma_start(
                        out=cc_buf[0, :, None],
                        out_offset=bass.IndirectOffsetOnAxis(ap=posi[:, j:j + 1],
                                                             axis=0),
                        in_=cv_all[:, j:j + 1], in_offset=None,
                        bounds_check=C - 1, oob_is_err=False)
                # gather this segment's tokens and stage them to DRAM
                # (overlaps with the following chunks' compute)
                for off, sz in ((0, P), (P, C8 - P)):
                    sb = pc * C8 + off
                    gidx = gpp.tile([P, 1], U32, tag="gidx")
                    nc.sync.dma_start(out=gidx[:sz], in_=idx_d[0, sb:sb + sz, None])
                    xg = gpp.tile([P, D], F32R, tag="xg", bufs=1)
                    nc.gpsimd.indirect_dma_start(
                        out=xg[:sz], out_offset=None, in_=x_d[:],
                        in_offset=bass.IndirectOffsetOnAxis(ap=gidx[:sz, 0:1], axis=0))
                    nc.sync.dma_start(out=xg_buf[sb:sb + sz, :], in_=xg[:sz])

            pending = None
            for c in range(NCH):
                cs = slice(c * TCH, (c + 1) * TCH)
                xf = xfp.tile([P, KD * TCH], F32, tag="xf")
                xf_v = xf[:].rearrange("p (k t) -> p k t", k=KD)
                nc.sync.dma_start(out=xf_v, in_=xT_r[:, :, cs])
                xr = xrp.tile([P, KD * TCH], F32R, tag="xr")
                xr_v = xr[:].rearrange("p (k t) -> p k t", k=KD)
                nc.gpsimd.tensor_copy(out=xr[:], in_=xf[:])

                # packed fp32 router: 4 col-groups, 4 k-tiles each
                rps = rps_p.tile([P, TCH], F32, tag="ra")
                for kk in range(4):
                    for j in range(4):
                        nc.tensor.matmul(rps[32 * j:32 * j + E, :],
                                         lhsT=wr_sb[4 * j + kk][:],
                                         rhs=xf_v[:, 4 * j + kk, :],
                                         tile_position=(0, 32 * j),
                                         start=(kk == 0), stop=(kk == 3))

                # previous chunk's position/scatter tail (inputs long ready)
                if pending is not None:
                    emit_pos_and_scatter(*pending)

                # shared expert gate/up matmuls, group 0
                pgu = []
                for m3 in range(3):
                    sz = SH_MS[m3]
                    msl = slice(m3 * P, m3 * P + sz)
                    pg = sp_p.tile([P, TCH], F32, tag="sp")
                    pu = sp_p.tile([P, TCH], F32, tag="sp")
                    for k in range(KD):
                        nc.tensor.matmul(pg[:sz], lhsT=wsg_v[:, k, msl],
                                         rhs=xr_v[:, k, :],
                                         start=(k == 0), stop=(k == KD - 1))
                    for k in range(KD):
                        nc.tensor.matmul(pu[:sz], lhsT=wsu_v[:, k, msl],
                                         rhs=xr_v[:, k, :],
                                         start=(k == 0), stop=(k == KD - 1))
                    pgu.append((pg, pu))
                    if m3 == 0:
                        # router combine rides between gate/up groups
                        for j in range(4):
                            nc.vector.tensor_copy(out=s4[32 * j:32 * j + E, :],
                                                  in_=rps[32 * j:32 * j + E, :])
                        cm = rps_p.tile([E, TCH], F32, tag="ra")
                        nc.tensor.matmul(cm[:], lhsT=m4_sb[:], rhs=s4[:],
                                         start=True, stop=True)
                        lgT = rout.tile([E, TCH], F32, tag="lgT")
                        nc.vector.tensor_copy(out=lgT[:], in_=cm[:])
                        exT = rout.tile([E, TCH], F32, tag="exT")
                        nc.scalar.activation(out=exT[:], in_=cm[:], func=AF.Exp)

                # shared SwiGLU evictions (DVE ahead of the softmax chain)
                hs = []
                for m3 in range(3):
                    sz = SH_MS[m3]
                    pg, pu = pgu[m3]
                    sg = hsp.tile([P, TCH], F32R, tag="sg")
                    nc.scalar.activation(out=sg[:sz], in_=pg[:sz], func=AF.Silu)
                    ht = hsp.tile([P, TCH], F32R, tag=f"hs{m3}", name=f"hs{m3}")
                    nc.vector.tensor_tensor(out=ht[:sz], in0=sg[:sz], in1=pu[:sz],
                                            op=AluOpType.mult)
                    hs.append(ht)

                # logit/exp transposes, then softmax chain (runs during down)
                m_all = rout.tile([P, MT], F32, tag="m_all")
                cv_all = rout.tile([P, MT], F32, tag="cv_all")
                lgexs = []
                for j in range(MT):
                    tps = rt_p.tile([P, 2 * E], F32, tag="rt")
                    nc.tensor.transpose(out=tps[:, :E],
                                        in_=lgT[:, j * P:(j + 1) * P],
                                        identity=identF[:E, :E])
                    nc.tensor.transpose(out=tps[:, E:],
                                        in_=exT[:, j * P:(j + 1) * P],
                                        identity=identF[:E, :E])
                    lgex = rout.tile([P, 2 * E], F32, tag=f"lgex{j}",
                                     name=f"lgex{j}")
                    nc.vector.tensor_copy(out=lgex[:], in_=tps[:])
                    lgexs.append(lgex)

                # shared down projection
                for mt in range(MT):
                    for n in range(ND):
                        py = yp_p.tile([P, 512], F32, tag="py")
                        for k3 in range(3):
                            sz = SH_MS[k3]
                            nc.tensor.matmul(
                                py[:], lhsT=hs[k3][:sz, mt * P:(mt + 1) * P],
                                rhs=wsd_sb[k3][:sz, n * 512:(n + 1) * 512],
                                start=(k3 == 0), stop=(k3 == 2))
                        ysb = ysp.tile([P, 512], F32, tag="ysb")
                        nc.vector.tensor_copy(out=ysb[:], in_=py[:])
                        nc.sync.dma_start(
                            out=y_d[c * TCH + mt * P: c * TCH + (mt + 1) * P,
                                    n * 512:(n + 1) * 512],
                            in_=ysb[:])

                for j in range(MT):
                    lgex = lgexs[j]
                    lg = lgex[:, :E]
                    ex = lgex[:, E:]
                    mx = rout.tile([P, E], F32, tag="mx")
                    nc.vector.max(out=mx[:], in_=lg)
                    selm = rout.tile([P, E], F32, tag="selm")
                    nc.vector.tensor_scalar(selm[:], lg, mx[:, 1:2], None,
                                            op0=AluOpType.is_ge)
                    mesel = rout.tile([P, E], F32, tag="mesel")
                    nc.vector.tensor_tensor(out=mesel[:], in0=selm[:],
                                            in1=esel_sb[:], op=AluOpType.mult)
                    nc.vector.reduce_sum(m_all[:, j:j + 1], mesel[:], axis=AX.X)
                    den = rout.tile([P, 1], F32, tag="den")
                    nc.vector.reduce_sum(den[:], ex, axis=AX.X)
                    rden = rout.tile([P, 1], F32, tag="rden")
                    nc.vector.reciprocal(rden[:], den[:])
                    prob = rout.tile([P, E], F32, tag="prob")
                    nc.vector.tensor_scalar(prob[:], ex, rden[:], None,
                                            op0=AluOpType.mult)
                    nc.vector.tensor_tensor(out=prob[:], in0=prob[:], in1=mesel[:],
                                            op=AluOpType.mult)
                    nc.vector.reduce_sum(cv_all[:, j:j + 1], prob[:], axis=AX.X)
                pending = (c, m_all, cv_all)

            emit_pos_and_scatter(*pending)

        # ---------------- phase 2: expert ----------------
        with ExitStack() as bctx:
            hTep = bctx.enter_context(tc.tile_pool(name="hTep", bufs=1))
            hTe = []
            for m in range(NME):
                t = hTep.tile([P, C], F32R, tag=f"hTe{m}", name=f"hTe{m}")
                hTe.append(t)

            with ExitStack() as b1ctx:
                xtep = b1ctx.enter_context(tc.tile_pool(name="xtep", bufs=1))
                xTe = xtep.tile([P, KD * C], F32R)
                cb = xtep.tile([P, C], F32R)
                xTe_r = xTe[:].rearrange("p (k c) -> p k c", k=KD)

                # 2a: gather + transpose
                with ExitStack() as cctx, nc.named_scope("gather"):
                    gp = cctx.enter_context(tc.tile_pool(name="gp", bufs=2))
                    crow = gp.tile([1, C], F32R, tag="crow", bufs=1)
                    nc.sync.dma_start(out=crow[:], in_=cc_buf[:].bitcast(F32R))
                    nc.gpsimd.partition_broadcast(cb[:], crow[:])
                    tp_p = cctx.enter_context(tc.tile_pool(name="tpp", bufs=3,
                                                           space="PSUM"))
                    so = 0
                    while so < C:
                        ssz = min(P, C - so)
                        xg = gp.tile([P, D], F32R, tag="xg", bufs=3)
                        nc.sync.dma_start(out=xg[:ssz], in_=xg_buf[so:so + ssz, :])
                        for k4 in range(KD // 4):
                            tp = tp_p.tile([P, 4 * P], F32R, tag="tp")
                            for kk in range(4):
                                k = k4 * 4 + kk
                                nc.tensor.transpose(out=tp[:, kk * P:kk * P + ssz],
                                                    in_=xg[:ssz, k * P:(k + 1) * P],
                                                    identity=identR[:ssz, :ssz])
                            nc.vector.tensor_copy(
                                out=xTe_r[:, k4 * 4:(k4 + 1) * 4, so:so + ssz],
                                in_=tp[:].rearrange("p (k c) -> p k c", k=4)[:, :, :ssz])
                        so += ssz

                # 2b: expert gate/up, SwiGLU * combine -> hTe (SBUF)
                with ExitStack() as dctx, nc.named_scope("p2b"):
                    wsp = dctx.enter_context(tc.tile_pool(name="wsp", bufs=1))
                    sp2 = dctx.enter_context(tc.tile_pool(name="sp2", bufs=4,
                                                          space="PSUM"))
                    hep = dctx.enter_context(tc.tile_pool(name="hep", bufs=2))
                    for m in range(NME):
                        msl = slice(m * P, (m + 1) * P)
                        wgm4, wum4 = [], []
                        for k4 in range(4):
                            t = wsp.tile([P, 4 * P], F32R, tag=f"wgm{k4}",
                                         name=f"wgm{k4}")
                            tv = t[:].rearrange("p (k m) -> p k m", k=4)
                            nc.sync.dma_start(
                                out=tv, in_=wg_r[:, 4 * k4:4 * (k4 + 1), msl])
                            wgm4.append(tv)
                        for k4 in range(4):
                            t = wsp.tile([P, 4 * P], F32R, tag=f"wum{k4}",
                                         name=f"wum{k4}")
                            tv = t[:].rearrange("p (k m) -> p k m", k=4)
                            nc.sync.dma_start(
                                out=tv, in_=wu_r[:, 4 * k4:4 * (k4 + 1), msl])
                            wum4.append(tv)
                        qo = 0
                        for q, qsz in enumerate(QS):
                            qsl = slice(qo, qo + qsz)
                            pg = sp2.tile([P, QS[0]], F32, tag="sp2")
                            pu = sp2.tile([P, QS[0]], F32, tag="sp2")
                            for k in range(KD):
                                nc.tensor.matmul(pg[:, :qsz],
                                                 lhsT=wgm4[k // 4][:, k % 4, :],
                                                 rhs=xTe_r[:, k, qsl],
                                                 start=(k == 0), stop=(k == KD - 1))
                            for k in range(KD):
                                nc.tensor.matmul(pu[:, :qsz],
                                                 lhsT=wum4[k // 4][:, k % 4, :],
                                                 rhs=xTe_r[:, k, qsl],
                                                 start=(k == 0), stop=(k == KD - 1))
                            sg = hep.tile([P, QS[0]], F32R, tag="sg2")
                            nc.scalar.activation(out=sg[:, :qsz], in_=pg[:, :qsz],
                                                 func=AF.Silu)
                            nc.vector.tensor_tensor(out=hTe[m][:, qsl], in0=sg[:, :qsz],
                                                    in1=pu[:, :qsz], op=AluOpType.mult)
                            nc.vector.tensor_tensor(out=hTe[m][:, qsl],
                                                    in0=hTe[m][:, qsl],
                                                    in1=cb[:, qsl], op=AluOpType.mult)
                            qo += qsz

            # 2c: expert down projection (weights streamed per n-chunk)
            with ExitStack() as ectx, nc.named_scope("p2c"):
                wdp = ectx.enter_context(tc.tile_pool(name="wdp", bufs=2))
                yp2 = ectx.enter_context(tc.tile_pool(name="yp2", bufs=3, space="PSUM"))
                yep = ectx.enter_context(tc.tile_pool(name="yep", bufs=3))
                for n in range(ND):
                    nsl = slice(n * 512, (n + 1) * 512)
                    wdn = wdp.tile([P, NME * 512], F32R, tag="wdn")
                    wdn_v = wdn[:].rearrange("p (k n) -> p k n", k=NME)
                    nc.sync.dma_start(out=wdn_v, in_=wd_r[:, :, nsl])
                    so = 0
                    while so < C:
                        ssz = min(P, C - so)
                        py = yp2.tile([P, 512], F32, tag="py2")
                        for k in range(NME):
                            nc.tensor.matmul(
                                py[:ssz], lhsT=hTe[k][:, so:so + ssz],
                                rhs=wdn_v[:, k, :],
                                start=(k == 0), stop=(k == NME - 1))
                        ysb = yep.tile([P, 512], F32, tag="ye_sb")
                        nc.vector.tensor_copy(out=ysb[:ssz], in_=py[:ssz])
                        nc.sync.dma_start(out=ye_d[so:so + ssz, nsl], in_=ysb[:ssz])
                        so += ssz

    nc.compile()
    return nc


def _get_program():
    if "nc" not in _CACHED:
        _CACHED["nc"] = _build_program()
    return _CACHED["nc"]


def kernel(x, W_router, We_gate, We_up, We_down, Ws_gate, Ws_up, Ws_down):
    x = np.asarray(x, np.float32)
    xf = x.reshape(T, D)
    xpad = np.zeros((T + 1, D), np.float32)
    xpad[:T] = xf
    xT = np.ascontiguousarray(xf.T)
    W_router = np.ascontiguousarray(np.asarray(W_router, np.float32))
    eye = np.eye(E, dtype=np.float32)
    ltri = np.triu(np.ones((P, P), np.float32), 0)  # L[q,p] = 1 if q <= p
    m4 = np.zeros((P, E), np.float32)
    for j in range(4):
        for m in range(E):
            m4[32 * j + m, m] = 1.0

    in_maps = []
    for e in range(E):
        sl = slice(e * DSH, (e + 1) * DSH)
        in_maps.append({
            "x": xpad,
            "xT": xT,
            "wg": np.ascontiguousarray(We_gate[e], np.float32),
            "wu": np.ascontiguousarray(We_up[e], np.float32),
            "wd": np.ascontiguousarray(We_down[e], np.float32),
            "wsg": np.ascontiguousarray(Ws_gate[:, sl], np.float32),
            "wsu": np.ascontiguousarray(Ws_up[:, sl], np.float32),
            "wsd": np.ascontiguousarray(Ws_down[sl, :], np.float32),
            "wr": W_router,
            "esel": np.tile(eye[e], (P, 1)),
            "ltri": ltri,
            "m4": m4,
        })

    nc = _get_program()
    trace = bool(int(os.environ.get("MOE_TRACE", "0")))
    res = run_bass_kernel_spmd(nc, in_maps, list(range(E)), trace=trace)
    if trace:
        _CACHED["last_results"] = res

    out = np.zeros((T, D), np.float64)
    acc = np.zeros((T + 1, D), np.float64)
    for e in range(E):
        out += res.results[e]["y"]
        idx = res.results[e]["idx"][0].astype(np.int64)
        acc[idx] += res.results[e]["ye"]
    out += acc[:T]
    return out.astype(np.float32).reshape(B, S, D)


# revision 21
# speedup vs baseline: 1.1842x; 1.1842x over previous
"""MoE kernel for 8-core TRN2 (Bass/Tile), expert-parallel with sparse
token dispatch.

Per core e (of 8):
  - Routed expert e computed SPARSELY: on-device fp32 top-2 routing
    builds a compact token list (capacity C=1600, 200 per 512-token
    chunk), tokens are gathered by indirect DMA, PE-transposed, and run
    through the expert FFN in float32r; compact outputs ye + token
    indices are returned and the host scatters them back.
  - Shared expert is tensor-parallel: core e owns columns/rows
    [e*352:(e+1)*352] of Ws_* and computes its dense partial y.
  - Router must match the fp32 reference top-2 selection, so it runs as
    fp32 matmuls, packed 4-wide into PE column groups (M=8 each) and
    combined with a small fp32 matmul.

Host: out = sum_e y_e  +  scatter_add_e(ye_e at idx_e).
"""

import os
from contextlib import ExitStack

import numpy as np

import concourse.bass as bass
import concourse.mybir as mybir
import concourse.tile as tile
from concourse import bacc
from concourse.alu_op_type import AluOpType
from concourse.bass_utils import run_bass_kernel_spmd
from concourse.masks import make_identity

F32 = mybir.dt.float32
F32R = mybir.dt.float32r
U32 = mybir.dt.uint32
AF = mybir.ActivationFunctionType
AX = mybir.AxisListType

P = 128
E = 8
D = 2048
DE = 1408
DS = 2816
DSH = DS // E            # 352
B, S = 2, 2048
T = B * S                # 4096

KD = D // P              # 16
TCH = 512
NCH = T // TCH           # 8
MT = TCH // P            # 4
ND = D // 512            # 4
SH_MS = [P, P, DSH - 2 * P]
NME = DE // P            # 11

C8 = 184                 # per-chunk expert capacity
C = C8 * NCH             # 1600
QS = [C // 4] * 4        # 400 each (>=256 keeps f32r at full rate)

_CACHED = {}


def _build_program():
    nc = bacc.Bacc("TRN2", target_bir_lowering=False, debug=False, num_devices=E)

    x_d = nc.dram_tensor("x", [T + 1, D], F32R, kind="ExternalInput")   # row T = 0
    xT_d = nc.dram_tensor("xT", [D, T], F32, kind="ExternalInput")
    xTr_d = nc.dram_tensor("xTr", [D, T], F32R, kind="ExternalInput")   # same data
    wg_d = nc.dram_tensor("wg", [D, DE], F32R, kind="ExternalInput")
    wu_d = nc.dram_tensor("wu", [D, DE], F32R, kind="ExternalInput")
    wd_d = nc.dram_tensor("wd", [DE, D], F32R, kind="ExternalInput")
    wsg_d = nc.dram_tensor("wsg", [D, DSH], F32R, kind="ExternalInput")
    wsu_d = nc.dram_tensor("wsu", [D, DSH], F32R, kind="ExternalInput")
    wsd_d = nc.dram_tensor("wsd", [DSH, D], F32R, kind="ExternalInput")
    wr_d = nc.dram_tensor("wr", [D, E], F32, kind="ExternalInput")
    esel_d = nc.dram_tensor("esel", [P, E], F32, kind="ExternalInput")
    ltri_d = nc.dram_tensor("ltri", [P, P], F32, kind="ExternalInput")  # L[q,p]=1 if q<=p
    m4_d = nc.dram_tensor("m4", [P, E], F32, kind="ExternalInput")      # col-group combine
    y_d = nc.dram_tensor("y", [T, D], F32, kind="ExternalOutput")
    ye_d = nc.dram_tensor("ye", [C, D], F32, kind="ExternalOutput")
    idx_d = nc.dram_tensor("idx", [1, C], U32, kind="ExternalOutput")

    xT_r = xT_d[:].rearrange("(k p) t -> p k t", p=P)
    xTr_r = xTr_d[:].rearrange("(k p) t -> p k t", p=P)
    wg_r = wg_d[:].rearrange("(k p) m -> p k m", p=P)
    wu_r = wu_d[:].rearrange("(k p) m -> p k m", p=P)
    wd_r = wd_d[:].rearrange("(k p) m -> p k m", p=P)

    with tile.TileContext(nc) as tc, ExitStack() as ctx:
        dram = ctx.enter_context(tc.tile_pool(name="dram", bufs=1, space="DRAM"))
        cc_buf = dram.tile([1, C], F32)
        xg_buf = dram.tile([C, D], F32R)

        const = ctx.enter_context(tc.tile_pool(name="const", bufs=1))
        identF = const.tile([P, P], F32)
        make_identity(nc, identF[:])
        identR = const.tile([P, P], F32R)
        nc.vector.tensor_copy(out=identR[:], in_=identF[:])
        esel_sb = const.tile([P, E], F32)
        nc.gpsimd.dma_start(out=esel_sb[:], in_=esel_d[:])
        ltri = const.tile([P, P], F32)
        nc.gpsimd.dma_start(out=ltri[:], in_=ltri_d[:])
        m4_sb = const.tile([P, E], F32)
        nc.gpsimd.dma_start(out=m4_sb[:], in_=m4_d[:])
        ones = const.tile([P, 1], F32)
        nc.vector.memset(ones[:], 1.0)
        wr_sb = []
        for k in range(KD):
            t = const.tile([P, E], F32, tag=f"wr{k}", name=f"wr{k}")
            nc.gpsimd.dma_start(out=t[:], in_=wr_d[k * P:(k + 1) * P, :])
            wr_sb.append(t)
        with tc.tile_pool(name="initp", bufs=1) as initp:
            initt = initp.tile([1, C], U32)
            nc.vector.memset(initt[:], T)
            nc.sync.dma_start(out=idx_d[:], in_=initt[:])
            initc = initp.tile([1, C], F32)
            nc.vector.memset(initc[:], 0.0)
            nc.sync.dma_start(out=cc_buf[:], in_=initc[:])
        tok_all = const.tile([P, T // P], U32)
        nc.gpsimd.iota(tok_all[:], pattern=[[P, T // P]], base=0, channel_multiplier=1)

        # ---------------- phase 1: routing + shared expert ----------------
        with ExitStack() as actx, nc.named_scope("phase1"):
            swp = actx.enter_context(tc.tile_pool(name="swp", bufs=1))
            wsg_sb = swp.tile([P, KD * DSH], F32R)
            wsg_v = wsg_sb[:].rearrange("p (k m) -> p k m", k=KD)
            nc.gpsimd.dma_start(out=wsg_v,
                                in_=wsg_d[:].rearrange("(k p) m -> p k m", p=P))
            wsu_sb = swp.tile([P, KD * DSH], F32R)
            wsu_v = wsu_sb[:].rearrange("p (k m) -> p k m", k=KD)
            nc.gpsimd.dma_start(out=wsu_v,
                                in_=wsu_d[:].rearrange("(k p) m -> p k m", p=P))
            wsd_sb = []
            for k3 in range(3):
                sz = SH_MS[k3]
                t = swp.tile([P, D], F32R, tag=f"wsd{k3}", name=f"wsd{k3}")
                nc.gpsimd.dma_start(out=t[:sz], in_=wsd_d[k3 * P:k3 * P + sz, :])
                wsd_sb.append(t)
            s4 = swp.tile([P, TCH], F32)
            nc.vector.memset(s4[:], 0.0)

            rps_p = actx.enter_context(tc.tile_pool(name="rps", bufs=1, space="PSUM"))
            rt_p = actx.enter_context(tc.tile_pool(name="rtp", bufs=2, space="PSUM"))
            pos_p = actx.enter_context(tc.tile_pool(name="posp", bufs=1, space="PSUM"))
            sp_p = actx.enter_context(tc.tile_pool(name="spp", bufs=2, space="PSUM"))
            yp_p = actx.enter_context(tc.tile_pool(name="ypp", bufs=2, space="PSUM"))
            xfp = actx.enter_context(tc.tile_pool(name="xfp", bufs=1))
            gpp = actx.enter_context(tc.tile_pool(name="gpp", bufs=2))
            # xg staging single-buffered (latency-tolerant)
            xrp = actx.enter_context(tc.tile_pool(name="xrp", bufs=2))
            rout = actx.enter_context(tc.tile_pool(name="rout", bufs=2))
            hsp = actx.enter_context(tc.tile_pool(name="hsp", bufs=2))
            ysp = actx.enter_context(tc.tile_pool(name="ysp", bufs=2))

            def emit_pos_and_scatter(pc, m_all, cv_all):
                """Positions + compact scatters for chunk pc (runs one chunk late
                so the PE-side ppre matmul never waits on the softmax chain)."""
                ppre = pos_p.tile([P, 2 * MT], F32, tag="ppre")
                nc.tensor.matmul(ppre[:, :MT], lhsT=ltri[:], rhs=m_all[:],
                                 start=True, stop=True)
                nc.tensor.matmul(ppre[:1, MT:], lhsT=ones[:], rhs=m_all[:],
                                 start=True, stop=True)
                pose = rout.tile([P, MT], F32, tag="pose")
                nc.vector.tensor_tensor(out=pose[:], in0=ppre[:, :MT], in1=m_all[:],
                                        op=AluOpType.subtract)
                cnt = rout.tile([1, MT], F32, tag="cnt")
                nc.vector.tensor_copy(out=cnt[:], in_=ppre[0:1, MT:])
                zero1 = rout.tile([1, MT], F32, tag="zero1")
                nc.vector.memset(zero1[:], 0.0)
                incl = rout.tile([1, MT], F32, tag="incl")
                nc.vector.tensor_tensor_scan(incl[:], cnt[:], zero1[:], 0.0,
                                             op0=AluOpType.add, op1=AluOpType.add)
                base = rout.tile([1, MT], F32, tag="base")
                nc.vector.tensor_sub(base[:], incl[:], cnt[:])
                base_b = rout.tile([P, MT], F32, tag="base_b")
                nc.gpsimd.partition_broadcast(base_b[:], base[:])
                nc.vector.tensor_add(pose[:], pose[:], base_b[:])
                pmask = rout.tile([P, MT], F32, tag="pmask")
                nc.vector.tensor_scalar(pmask[:], m_all[:], float(-C),
                                        float(C + pc * C8),
                                        op0=AluOpType.mult, op1=AluOpType.add)
                nc.vector.tensor_add(pmask[:], pmask[:], pose[:])
                posi = rout.tile([P, MT], U32, tag="posi")
                nc.vector.tensor_copy(out=posi[:], in_=pmask[:])
                for j in range(MT):
                    nc.gpsimd.indirect_dma_start(
                        out=idx_d[0, :, None],
                        out_offset=bass.IndirectOffsetOnAxis(ap=posi[:, j:j + 1],
                                                             axis=0),
                        in_=tok_all[:, pc * MT + j:pc * MT + j + 1], in_offset=None,
                        bounds_check=C - 1, oob_is_err=False)
                    nc.gpsimd.indirect_dma_start(
                        out=cc_buf[0, :, None],
                        out_offset=bass.IndirectOffsetOnAxis(ap=posi[:, j:j + 1],
                                                             axis=0),
                        in_=cv_all[:, j:j + 1], in_offset=None,
                        bounds_check=C - 1, oob_is_err=False)
                # gather this segment's tokens and stage them to DRAM
                # (overlaps with the following chunks' compute)
                for off, sz in ((0, P), (P, C8 - P)):
                    sb = pc * C8 + off
                    gidx = gpp.tile([P, 1], U32, tag="gidx")
                    nc.sync.dma_start(out=gidx[:sz], in_=idx_d[0, sb:sb + sz, None])
                    xg = gpp.tile([P, D], F32R, tag="xg", bufs=1)
                    nc.gpsimd.indirect_dma_start(
                        out=xg[:sz], out_offset=None, in_=x_d[:],
                        in_offset=bass.IndirectOffsetOnAxis(ap=gidx[:sz, 0:1], axis=0))
                    nc.sync.dma_start(out=xg_buf[sb:sb + sz, :], in_=xg[:sz])

            pending = None
            for c in range(NCH):
                cs = slice(c * TCH, (c + 1) * TCH)
                xf = xfp.tile([P, KD * TCH], F32, tag="xf")
                xf_v = xf[:].rearrange("p (k t) -> p k t", k=KD)
                nc.sync.dma_start(out=xf_v, in_=xT_r[:, :, cs])
                xr = xrp.tile([P, KD * TCH], F32R, tag="xr")
                xr_v = xr[:].rearrange("p (k t) -> p k t", k=KD)
                nc.sync.dma_start(out=xr_v, in_=xTr_r[:, :, cs])

                # packed fp32 router: 4 col-groups, 4 k-tiles each
                rps = rps_p.tile([P, TCH], F32, tag="ra")
                for kk in range(4):
                    for j in range(4):
                        nc.tensor.matmul(rps[32 * j:32 * j + E, :],
                                         lhsT=wr_sb[4 * j + kk][:],
                                         rhs=xf_v[:, 4 * j + kk, :],
                                         tile_position=(0, 32 * j),
                                         start=(kk == 0), stop=(kk == 3))

                # previous chunk's position/scatter tail (inputs long ready)
                if pending is not None:
                    emit_pos_and_scatter(*pending)

                # shared expert gate/up matmuls, group 0
                pgu = []
                for m3 in range(3):
                    sz = SH_MS[m3]
                    msl = slice(m3 * P, m3 * P + sz)
                    pg = sp_p.tile([P, TCH], F32, tag="sp")
                    pu = sp_p.tile([P, TCH], F32, tag="sp")
                    for k in range(KD):
                        nc.tensor.matmul(pg[:sz], lhsT=wsg_v[:, k, msl],
                                         rhs=xr_v[:, k, :],
                                         start=(k == 0), stop=(k == KD - 1))
                    for k in range(KD):
                        nc.tensor.matmul(pu[:sz], lhsT=wsu_v[:, k, msl],
                                         rhs=xr_v[:, k, :],
                                         start=(k == 0), stop=(k == KD - 1))
                    pgu.append((pg, pu))
                    if m3 == 0:
                        # router combine rides between gate/up groups
                        for j in range(4):
                            nc.vector.tensor_copy(out=s4[32 * j:32 * j + E, :],
                                                  in_=rps[32 * j:32 * j + E, :])
                        cm = rps_p.tile([E, TCH], F32, tag="ra")
                        nc.tensor.matmul(cm[:], lhsT=m4_sb[:], rhs=s4[:],
                                         start=True, stop=True)
                        lgT = rout.tile([E, TCH], F32, tag="lgT")
                        nc.vector.tensor_copy(out=lgT[:], in_=cm[:])
                        exT = rout.tile([E, TCH], F32, tag="exT")
                        nc.scalar.activation(out=exT[:], in_=cm[:], func=AF.Exp)

                # shared SwiGLU evictions (DVE ahead of the softmax chain)
                hs = []
                for m3 in range(3):
                    sz = SH_MS[m3]
                    pg, pu = pgu[m3]
                    sg = hsp.tile([P, TCH], F32R, tag="sg")
                    nc.scalar.activation(out=sg[:sz], in_=pg[:sz], func=AF.Silu)
                    ht = hsp.tile([P, TCH], F32R, tag=f"hs{m3}", name=f"hs{m3}")
                    nc.vector.tensor_tensor(out=ht[:sz], in0=sg[:sz], in1=pu[:sz],
                                            op=AluOpType.mult)
                    hs.append(ht)

                # logit/exp transposes, then softmax chain (runs during down)
                m_all = rout.tile([P, MT], F32, tag="m_all")
                cv_all = rout.tile([P, MT], F32, tag="cv_all")
                lgexs = []
                for j in range(MT):
                    tps = rt_p.tile([P, 2 * E], F32, tag="rt")
                    nc.tensor.transpose(out=tps[:, :E],
                                        in_=lgT[:, j * P:(j + 1) * P],
                                        identity=identF[:E, :E])
                    nc.tensor.transpose(out=tps[:, E:],
                                        in_=exT[:, j * P:(j + 1) * P],
                                        identity=identF[:E, :E])
                    lgex = rout.tile([P, 2 * E], F32, tag=f"lgex{j}",
                                     name=f"lgex{j}")
                    nc.vector.tensor_copy(out=lgex[:], in_=tps[:])
                    lgexs.append(lgex)

                # shared down projection
                for mt in range(MT):
                    for n in range(ND):
                        py = yp_p.tile([P, 512], F32, tag="py")
                        for k3 in range(3):
                            sz = SH_MS[k3]
                            nc.tensor.matmul(
                                py[:], lhsT=hs[k3][:sz, mt * P:(mt + 1) * P],
                                rhs=wsd_sb[k3][:sz, n * 512:(n + 1) * 512],
                                start=(k3 == 0), stop=(k3 == 2))
                        ysb = ysp.tile([P, 512], F32, tag="ysb")
                        nc.vector.tensor_copy(out=ysb[:], in_=py[:])
                        nc.sync.dma_start(
                            out=y_d[c * TCH + mt * P: c * TCH + (mt + 1) * P,
                                    n * 512:(n + 1) * 512],
                            in_=ysb[:])

                for j in range(MT):
                    lgex = lgexs[j]
                    lg = lgex[:, :E]
                    ex = lgex[:, E:]
                    mx = rout.tile([P, E], F32, tag="mx")
                    nc.vector.max(out=mx[:], in_=lg)
                    selm = rout.tile([P, E], F32, tag="selm")
                    nc.vector.tensor_scalar(selm[:], lg, mx[:, 1:2], None,
                                            op0=AluOpType.is_ge)
                    mesel = rout.tile([P, E], F32, tag="mesel")
                    nc.vector.tensor_tensor(out=mesel[:], in0=selm[:],
                                            in1=esel_sb[:], op=AluOpType.mult)
                    nc.vector.reduce_sum(m_all[:, j:j + 1], mesel[:], axis=AX.X)
                    den = rout.tile([P, 1], F32, tag="den")
                    nc.vector.reduce_sum(den[:], ex, axis=AX.X)
                    rden = rout.tile([P, 1], F32, tag="rden")
                    nc.vector.reciprocal(rden[:], den[:])
                    prob = rout.tile([P, E], F32, tag="prob")
                    nc.vector.tensor_scalar(prob[:], ex, rden[:], None,
                                            op0=AluOpType.mult)
                    nc.vector.tensor_tensor(out=prob[:], in0=prob[:], in1=mesel[:],
                                            op=AluOpType.mult)
                    nc.vector.reduce_sum(cv_all[:, j:j + 1], prob[:], axis=AX.X)
                pending = (c, m_all, cv_all)

            emit_pos_and_scatter(*pending)

        # ---------------- phase 2: expert ----------------
        with ExitStack() as bctx:
            hTep = bctx.enter_context(tc.tile_pool(name="hTep", bufs=1))
            hTe = []
            for m in range(NME):
                t = hTep.tile([P, C], F32R, tag=f"hTe{m}", name=f"hTe{m}")
                hTe.append(t)

            with ExitStack() as b1ctx:
                xtep = b1ctx.enter_context(tc.tile_pool(name="xtep", bufs=1))
                xTe = xtep.tile([P, KD * C], F32R)
                cb = xtep.tile([P, C], F32R)
                xTe_r = xTe[:].rearrange("p (k c) -> p k c", k=KD)

                # 2a: gather + transpose
                with ExitStack() as cctx, nc.named_scope("gather"):
                    gp = cctx.enter_context(tc.tile_pool(name="gp", bufs=2))
                    crow = gp.tile([1, C], F32R, tag="crow", bufs=1)
                    nc.sync.dma_start(out=crow[:], in_=cc_buf[:].bitcast(F32R))
                    nc.gpsimd.partition_broadcast(cb[:], crow[:])
                    tp_p = cctx.enter_context(tc.tile_pool(name="tpp", bufs=3,
                                                           space="PSUM"))
                    so = 0
                    while so < C:
                        ssz = min(P, C - so)
                        xg = gp.tile([P, D], F32R, tag="xg", bufs=3)
                        nc.sync.dma_start(out=xg[:ssz], in_=xg_buf[so:so + ssz, :])
                        for k4 in range(KD // 4):
                            tp = tp_p.tile([P, 4 * P], F32R, tag="tp")
                            for kk in range(4):
                                k = k4 * 4 + kk
                                nc.tensor.transpose(out=tp[:, kk * P:kk * P + ssz],
                                                    in_=xg[:ssz, k * P:(k + 1) * P],
                                                    identity=identR[:ssz, :ssz])
                            nc.vector.tensor_copy(
                                out=xTe_r[:, k4 * 4:(k4 + 1) * 4, so:so + ssz],
                                in_=tp[:].rearrange("p (k c) -> p k c", k=4)[:, :, :ssz])
                        so += ssz

                # 2b: expert gate/up, SwiGLU * combine -> hTe (SBUF)
                with ExitStack() as dctx, nc.named_scope("p2b"):
                    wsp = dctx.enter_context(tc.tile_pool(name="wsp", bufs=1))
                    sp2 = dctx.enter_context(tc.tile_pool(name="sp2", bufs=4,
                                                          space="PSUM"))
                    hep = dctx.enter_context(tc.tile_pool(name="hep", bufs=2))
                    for m in range(NME):
                        msl = slice(m * P, (m + 1) * P)
                        wgm4, wum4 = [], []
                        for k4 in range(4):
                            t = wsp.tile([P, 4 * P], F32R, tag=f"wgm{k4}",
                                         name=f"wgm{k4}")
                            tv = t[:].rearrange("p (k m) -> p k m", k=4)
                            nc.sync.dma_start(
                                out=tv, in_=wg_r[:, 4 * k4:4 * (k4 + 1), msl])
                            wgm4.append(tv)
                        for k4 in range(4):
                            t = wsp.tile([P, 4 * P], F32R, tag=f"wum{k4}",
                                         name=f"wum{k4}")
                            tv = t[:].rearrange("p (k m) -> p k m", k=4)
                            nc.sync.dma_start(
                                out=tv, in_=wu_r[:, 4 * k4:4 * (k4 + 1), msl])
                            wum4.append(tv)
                        qo = 0
                        for q, qsz in enumerate(QS):
                            qsl = slice(qo, qo + qsz)
                            pg = sp2.tile([P, QS[0]], F32, tag="sp2")
                            pu = sp2.tile([P, QS[0]], F32, tag="sp2")
                            for k in range(KD):
                                nc.tensor.matmul(pg[:, :qsz],
                                                 lhsT=wgm4[k // 4][:, k % 4, :],
                                                 rhs=xTe_r[:, k, qsl],
                                                 start=(k == 0), stop=(k == KD - 1))
                            for k in range(KD):
                                nc.tensor.matmul(pu[:, :qsz],
                                                 lhsT=wum4[k // 4][:, k % 4, :],
                                                 rhs=xTe_r[:, k, qsl],
                                                 start=(k == 0), stop=(k == KD - 1))
                            sg = hep.tile([P, QS[0]], F32R, tag="sg2")
                            nc.scalar.activation(out=sg[:, :qsz], in_=pg[:, :qsz],
                                                 func=AF.Silu)
                            nc.vector.tensor_tensor(out=hTe[m][:, qsl], in0=sg[:, :qsz],
                                                    in1=pu[:, :qsz], op=AluOpType.mult)
                            nc.vector.tensor_tensor(out=hTe[m][:, qsl],
                                                    in0=hTe[m][:, qsl],
                                                    in1=cb[:, qsl], op=AluOpType.mult)
                            qo += qsz

            # 2c: expert down projection (weights streamed per n-chunk)
            with ExitStack() as ectx, nc.named_scope("p2c"):
                wdp = ectx.enter_context(tc.tile_pool(name="wdp", bufs=2))
                yp2 = ectx.enter_context(tc.tile_pool(name="yp2", bufs=3, space="PSUM"))
                yep = ectx.enter_context(tc.tile_pool(name="yep", bufs=3))
                for n in range(ND):
                    nsl = slice(n * 512, (n + 1) * 512)
                    wdn = wdp.tile([P, NME * 512], F32R, tag="wdn")
                    wdn_v = wdn[:].rearrange("p (k n) -> p k n", k=NME)
                    nc.sync.dma_start(out=wdn_v, in_=wd_r[:, :, nsl])
                    so = 0
                    while so < C:
                        ssz = min(P, C - so)
                        py = yp2.tile([P, 512], F32, tag="py2")
                        for k in range(NME):
                            nc.tensor.matmul(
                                py[:ssz], lhsT=hTe[k][:, so:so + ssz],
                                rhs=wdn_v[:, k, :],
                                start=(k == 0), stop=(k == NME - 1))
                        ysb = yep.tile([P, 512], F32, tag="ye_sb")
                        nc.vector.tensor_copy(out=ysb[:ssz], in_=py[:ssz])
                        nc.sync.dma_start(out=ye_d[so:so + ssz, nsl], in_=ysb[:ssz])
                        so += ssz

    nc.compile()
    return nc


def _get_program():
    if "nc" not in _CACHED:
        _CACHED["nc"] = _build_program()
    return _CACHED["nc"]


def kernel(x, W_router, We_gate, We_up, We_down, Ws_gate, Ws_up, Ws_down):
    x = np.asarray(x, np.float32)
    xf = x.reshape(T, D)
    xpad = np.zeros((T + 1, D), np.float32)
    xpad[:T] = xf
    xT = np.ascontiguousarray(xf.T)
    W_router = np.ascontiguousarray(np.asarray(W_router, np.float32))
    eye = np.eye(E, dtype=np.float32)
    ltri = np.triu(np.ones((P, P), np.float32), 0)  # L[q,p] = 1 if q <= p
    m4 = np.zeros((P, E), np.float32)
    for j in range(4):
        for m in range(E):
            m4[32 * j + m, m] = 1.0

    in_maps = []
    for e in range(E):
        sl = slice(e * DSH, (e + 1) * DSH)
        in_maps.append({
            "x": xpad,
            "xT": xT,
            "xTr": xT,
            "wg": np.ascontiguousarray(We_gate[e], np.float32),
            "wu": np.ascontiguousarray(We_up[e], np.float32),
            "wd": np.ascontiguousarray(We_down[e], np.float32),
            "wsg": np.ascontiguousarray(Ws_gate[:, sl], np.float32),
            "wsu": np.ascontiguousarray(Ws_up[:, sl], np.float32),
            "wsd": np.ascontiguousarray(Ws_down[sl, :], np.float32),
            "wr": W_router,
            "esel": np.tile(eye[e], (P, 1)),
            "ltri": ltri,
            "m4": m4,
        })

    nc = _get_program()
    trace = bool(int(os.environ.get("MOE_TRACE", "0")))
    res = run_bass_kernel_spmd(nc, in_maps, list(range(E)), trace=trace)
    if trace:
        _CACHED["last_results"] = res

    out = np.zeros((T, D), np.float64)
    acc = np.zeros((T + 1, D), np.float64)
    for e in range(E):
        out += res.results[e]["y"]
        idx = res.results[e]["idx"][0].astype(np.int64)
        acc[idx] += res.results[e]["ye"]
    out += acc[:T]
    return out.astype(np.float32).reshape(B, S, D)
